# revision 21
# baseline (speedup 1.0000x reference)
"""MoE layer (top-2 of 8 experts) for 8 Trainium2 NeuronCores.

Strategy: expert-parallel. Host computes the (tiny) router + top-2 dispatch in
numpy; core e runs expert e's FFN over its dispatched tokens (padded to a fixed
capacity C); host combines the two expert outputs per token.

All device matmuls are [K=128]x[M=128]x[N=512] bf16 (1 cycle/row, fp32 PSUM).
bf16 beats fp32r here: same 1 cyc/row stream rate, but fast-weight-load keeps
the per-matmul 128-cycle LDWEIGHTS hidden (fp32r has no FWL and measures ~25%
slower end-to-end), and input DMA bytes halve. max-rel error ~4e-3 vs the
fp32 reference (gate 2e-2).
  gate^T/up^T [H, Ct] = gwT/uwT.T @ xt   (contraction over D, 8 k-tiles)
  h = silu(gate) * up                    (SBUF-resident [128, 512] bf16 tiles)
  y [Ct, D] = (h.T @ dwT) * p            (contraction over H, 16 k-tiles,
                                          combine-prob scale fused in eviction)

Host pre-packs weights/tokens into SBUF-tile order so every load is one large
DMA with >=1KB contiguous lines. Queue layout: weights on the two HWDGE queues
(gate+down on sync, up+x+p on scalar), y stores alone on the gpsimd/SWDGE
queue so they never delay the next superblock's x loads. Stage-B evictions
alternate scalar/DVE and stores go out in half-batches so the end-of-body
DMA-quiesce tail stays short. y is written bf16 (halves the tail transfer).

Second session: the default body is now emit_expert_ffn_v3 — identical
instruction stream to the original, but superblock 1 iterates ht4/dc in
reverse and reuses the gate/up tiles of ht4 {3,2} and the dc=1 down tiles
still resident in SBUF from superblock 0 (wpool/dpool bufs 3->4), cutting
per-rep weight DMA 24->18MB. Paired ABBA measurement vs the original:
-11us median (graded LO2/HI34 protocol, n=30), -22us median in the deep-hot
regime (LO66/HI130, n=12, IQR fully negative) — the win is power/thermal:
fewer HBM bytes at identical compute. Full reuse (wpool/dpool bufs=8, zero
sb1 reloads, 14MB/rep) measured tie-or-worse than partial reuse — the
bytes->time benefit saturates; don't assume monotonicity.

Optimization notes (second session): this body is at the bf16 PE roofline.
1536 N=512 matmuls = 786k PE cycles = 327.7us @ 2.4GHz; the measured For_i
marginal per-rep at short bursts is 328.1us (reps pipeline across the loop
back-edge; TimelineSim agrees: 328.7us marginal, PE 92.7% busy, LDWEIGHTS
fully hidden). The reported ~380-400us/rep number is power-bound, not
structure-bound: sustained bursts throttle the PE to ~1.9GHz (P0), where the
roofline is ~410us; the LO=2/HI=34 protocol straddles the transition and
run-to-run thermal state dominates (+-15us for identical binaries). Variants
tried and rejected as no-better-or-worse under paired measurement: HWDGE
full-tile stores (sync/scalar rings), staggered_reset For_i, a
single-superblock body that halves gate/up weight DMA (16->8MB/rep; equal
cold, reproducibly ~8-13us WORSE in the deep-hot regime), fp8 matmuls
(max-rel 6.6e-2 >> 2e-2 budget; DoubleRow also loses FWL below FD>=256),
and bf16-PSUM N=1024 matmuls (TRN2 requires fp32 matmul output; TRN3-only).
"""

import ml_dtypes
import numpy as np

import concourse.bass as bass
import concourse.mybir as mybir
import concourse.tile as tile
from concourse import bacc
from concourse.bass_utils import run_bass_kernel_spmd

E = 8
TOP_K = 2
B, S, D, H = 4, 2048, 1024, 2048
T = B * S
C = 2048          # per-expert token capacity; overflow pairs (seed-0: ~137
                  # of 16384, counts max 2175) fall back to exact host compute
CT = 512          # token tile
P = 128
NCT = C // CT     # 4
KD = D // P       # 8  k-tiles for gate/up
KH = H // P       # 16 k-tiles for down
NH4 = H // 512    # 4  groups of 4 h-blocks
F32 = mybir.dt.float32
F32R = mybir.dt.float32r
BF16 = mybir.dt.bfloat16
NPBF16 = ml_dtypes.bfloat16
AF = mybir.ActivationFunctionType


def emit_expert_ffn(tc, xt, gw, uw, dw, pv, y, dma_in=True, dma_out=True,
                    micro=True, store_mode="gpsimd_half"):
    """Emit one expert's FFN.

    DRAM tensors (all pre-packed on host):
      xt [NCT, 128, KD, 512] bf16 - tokens, transposed per ct tile
      gw/uw [NH4, 128, KD, 512] bf16 - gate/up weights per 4-h-block group
      dw [2, 2, 128, KH//2, 512] bf16 - down weights per (dc, kh-half)
      pv [128, C//128] f32 - combine probs (token-partition layout)
      y  [NCT, 2, 128, 4, 512] bf16 out - [ct, dc, p, m, 512]

    dma_in/dma_out: timing-experiment knobs that drop the input/output DMAs
    (compute runs on whatever is in SBUF); both True for the real kernel.
    """
    nc = tc.nc

    def dma(engine, dst, src):
        if dma_in:
            engine.dma_start(dst, src)
        else:
            # timing experiment: 1-element-per-partition DMA keeps the tile
            # written (scheduler requirement) with negligible HBM traffic
            ix = (slice(None),) + (slice(0, 1),) * (len(dst.shape) - 1)
            engine.dma_start(dst[ix], src[ix])
    # superblocks of up to 2 token tiles sharing one weight pass
    sbs = [list(range(s, min(s + 2, NCT))) for s in range(0, NCT, 2)]

    with (
        tc.tile_pool(name="xpool", bufs=2) as xpool,
        tc.tile_pool(name="wpool", bufs=3) as wpool,
        tc.tile_pool(name="hpool", bufs=36) as hpool,
        tc.tile_pool(name="dpool", bufs=3) as dpool,
        tc.tile_pool(name="tpool", bufs=5) as tpool,
        tc.tile_pool(name="opool", bufs=2) as opool,
        tc.tile_pool(name="ppool", bufs=1) as ppool,
        tc.tile_pool(name="pspool", bufs=8, space="PSUM") as pspool,
    ):
        p_sb = ppool.tile([P, C // P], F32)
        dma(nc.scalar, p_sb[:, :], pv[:, :])

        for cts in sbs:
            # ---- token tiles ----
            # on the scalar (HWDGE) queue: the gpsimd/SWDGE queue carries the
            # y stores, which would delay the next superblock's x loads by
            # ~10us of descriptor-gen right when the PE needs them
            first_sb = cts[0] == 0
            xts = []
            x1_pending = None
            for ct in cts:
                x_t = xpool.tile([P, KD, CT], BF16, name=f"xt_{ct}", tag="xt")
                if ct == 0:
                    # k-slice granularity on the first tile: the first matmul
                    # group consumes k-slices in order, so it can start after
                    # slice 0 instead of after the full half-tile
                    g0 = 1 if micro else 2
                    for q in range(KD // g0):
                        dma(nc.scalar, x_t[:, g0 * q:g0 * q + g0, :],
                            xt[ct][:, g0 * q:g0 * q + g0, :])
                elif first_sb:
                    # defer x1's load into the ht4==0 block, between the two
                    # halves of the up-weight load (both on the scalar queue):
                    # u0's first half then arrives before the first up-matmul
                    # group needs it, x1 before the ci=1 gate group
                    x1_pending = (x_t, ct)
                else:
                    dma(nc.scalar, x_t[:, :, :], xt[ct][:, :, :])
                xts.append(x_t)
            hs = [
                [
                    hpool.tile([P, CT], BF16, name=f"h_{ct}_{ht}", tag="h")
                    for ht in range(KH)
                ]
                for ct in cts
            ]

            # ---- stage A: gate/up matmuls + silu*mul -> h ----
            for ht4 in range(NH4):
                gt = wpool.tile([P, KD, 512], BF16, name=f"g_{ht4}", tag="w")
                if ht4 == 0 and first_sb:
                    # quarter-granularity on the very first load so the first
                    # matmuls start ~2us earlier out of the cold start
                    g0 = 1 if micro else 2
                    for q in range(KD // g0):
                        dma(nc.sync,
                            gt[:, g0 * q:g0 * q + g0, :],
                            gw[ht4][:, g0 * q:g0 * q + g0, :]
                        )
                else:
                    dma(nc.sync, gt[:, :, :], gw[ht4][:, :, :])
                ut = wpool.tile([P, KD, 512], BF16, name=f"u_{ht4}", tag="w")
                if ht4 == 0 and x1_pending is not None:
                    x1_t, x1_ct = x1_pending
                    dma(nc.scalar, ut[:, 0:4, :], uw[ht4][:, 0:4, :])
                    dma(nc.scalar, x1_t[:, :, :], xt[x1_ct][:, :, :])
                    dma(nc.scalar, ut[:, 4:8, :], uw[ht4][:, 4:8, :])
                else:
                    dma(nc.scalar, ut[:, :, :], uw[ht4][:, :, :])
                # ct-major, all-gate-then-all-up: gt's last use lands at ~75%
                # of the group so the next group's weight DMA overlaps compute
                for ci in range(len(cts)):
                    tmps = []
                    for sub in range(4):
                        ht = ht4 * 4 + sub
                        pg = pspool.tile([P, CT], F32, name=f"pg_{ht}_{ci}", tag="ps")
                        for kt in range(KD):
                            nc.tensor.matmul(
                                pg[:, :],
                                gt[:, kt, sub * P:(sub + 1) * P],
                                xts[ci][:, kt, :],
                                start=(kt == 0),
                                stop=(kt == KD - 1),
                            )
                        tmp = tpool.tile([P, CT], F32, name=f"t_{ht}_{ci}", tag="t")
                        nc.scalar.activation(tmp[:, :], pg[:, :], AF.Silu)
                        tmps.append(tmp)
                    for sub in range(4):
                        ht = ht4 * 4 + sub
                        pu = pspool.tile([P, CT], F32, name=f"pu_{ht}_{ci}", tag="ps")
                        for kt in range(KD):
                            nc.tensor.matmul(
                                pu[:, :],
                                ut[:, kt, sub * P:(sub + 1) * P],
                                xts[ci][:, kt, :],
                                start=(kt == 0),
                                stop=(kt == KD - 1),
                            )
                        nc.vector.tensor_mul(
                            hs[ci][ht][:, :], tmps[sub][:, :], pu[:, :]
                        )

            # ---- stage B: down matmuls + prob scale -> y ----
            for dc in range(2):
                pos = {}
                for ci in range(len(cts)):
                    for m in range(CT // P):
                        pos[(ci, m)] = pspool.tile(
                            [P, 512], F32, name=f"po_{dc}_{ci}_{m}", tag="ps"
                        )
                ots = [
                    opool.tile([P, CT // P, 512], BF16,
                               name=f"o_{dc}_{ci}", tag="o")
                    for ci in range(len(cts))
                ]
                for hf in range(4):
                    dt_ = dpool.tile([P, KH // 4, 512], BF16, name=f"d_{dc}_{hf}", tag="dw")
                    dma(nc.sync,
                        dt_[:, :, :], dw[dc, hf // 2][:, (hf % 2) * 4:(hf % 2) * 4 + 4, :]
                    )
                    for kb in range(KH // 4):
                        kh = hf * (KH // 4) + kb
                        for ci in range(len(cts)):
                            for m in range(CT // P):
                                nc.tensor.matmul(
                                    pos[(ci, m)][:, :],
                                    hs[ci][kh][:, m * P:(m + 1) * P],
                                    dt_[:, kb, :],
                                    start=(kh == 0),
                                    stop=(kh == KH - 1),
                                )
                for ci, ct in enumerate(cts):
                    for m in range(CT // P):
                        j = ct * (CT // P) + m
                        # split evictions across scalar/DVE so the post-last-
                        # matmul drain chain is half as long
                        if m % 2 == 0:
                            nc.scalar.mul(
                                ots[ci][:, m, :], pos[(ci, m)][:, :], p_sb[:, j:j + 1]
                            )
                        else:
                            nc.vector.tensor_scalar_mul(
                                ots[ci][:, m, :], pos[(ci, m)][:, :], p_sb[:, j:j + 1]
                            )
                        # store policy; see store_mode in build_nc
                        last_round = micro and dc == 1 and cts[-1] == NCT - 1
                        if not dma_out:
                            pass
                        elif store_mode == "gpsimd_half":
                            # half-batched SWDGE stores (16/body)
                            if last_round:
                                nc.gpsimd.dma_start(
                                    y[ct, dc][:, m:m + 1, :],
                                    ots[ci][:, m:m + 1, :],
                                )
                            elif m % 2 == 1:
                                nc.gpsimd.dma_start(
                                    y[ct, dc][:, m - 1:m + 1, :],
                                    ots[ci][:, m - 1:m + 1, :],
                                )
                        elif store_mode in ("sync_full", "split_full"):
                            # one 512KB HWDGE store per (ct, dc) after all 4
                            # evictions; last round splits across both HWDGE
                            # rings so the end-of-body tail is half as long
                            if last_round and m % 2 == 1:
                                eng = nc.sync if m == 1 else nc.scalar
                                eng.dma_start(
                                    y[ct, dc][:, m - 1:m + 1, :],
                                    ots[ci][:, m - 1:m + 1, :],
                                )
                            elif m == CT // P - 1:
                                eng = (
                                    nc.sync
                                    if (store_mode == "sync_full" or dc == 0)
                                    else nc.scalar
                                )
                                eng.dma_start(
                                    y[ct, dc][:, :, :], ots[ci][:, :, :]
                                )
                        elif store_mode == "sync_half":
                            if last_round:
                                eng = nc.sync if m % 2 == 0 else nc.scalar
                                eng.dma_start(
                                    y[ct, dc][:, m:m + 1, :],
                                    ots[ci][:, m:m + 1, :],
                                )
                            elif m % 2 == 1:
                                nc.sync.dma_start(
                                    y[ct, dc][:, m - 1:m + 1, :],
                                    ots[ci][:, m - 1:m + 1, :],
                                )
                        else:
                            raise ValueError(store_mode)


def emit_expert_ffn_v2(tc, xt, gw, uw, dw, pv, y, dma_in=True, dma_out=True,
                       micro=True, store_mode="split_full"):
    """Single-superblock body: all 4 token tiles share one weight pass, so
    gate/up weights stream from HBM once per rep (8MB) instead of twice
    (16MB), and down weights are SBUF-resident for the whole stage B.

    Stage A: per ht4 group (4 h-blocks of 128), for each of 4 token tiles:
      gate psum x4 -> silu -> tmp; up psum x4 -> h = tmp*up.
    Stage B: dw fully loaded up front; for dc in 2, for ci-pair in
      [(0,1),(2,3)]: 8 psum banks accumulate over all 16 kh tiles, evict
      with the combine-prob scale fused, store per (ct, dc) as one 512KB
      HWDGE transfer (dc0 on sync ring, dc1 on scalar ring).
    """
    nc = tc.nc

    def dma(engine, dst, src):
        if dma_in:
            engine.dma_start(dst, src)
        else:
            ix = (slice(None),) + (slice(0, 1),) * (len(dst.shape) - 1)
            engine.dma_start(dst[ix], src[ix])

    cts = list(range(NCT))
    with (
        tc.tile_pool(name="xpool", bufs=4) as xpool,
        tc.tile_pool(name="wpool", bufs=3) as wpool,
        tc.tile_pool(name="hpool", bufs=64) as hpool,
        tc.tile_pool(name="dpool", bufs=8) as dpool,
        tc.tile_pool(name="tpool", bufs=5) as tpool,
        tc.tile_pool(name="opool", bufs=4) as opool,
        tc.tile_pool(name="ppool", bufs=1) as ppool,
        tc.tile_pool(name="pspool", bufs=8, space="PSUM") as pspool,
    ):
        p_sb = ppool.tile([P, C // P], F32)
        dma(nc.scalar, p_sb[:, :], pv[:, :])

        # ---- token tiles: x0 micro-sliced for fast cold start; x1 deferred
        # between the two halves of u0's load; x2/x3 deferred after u0 so
        # the scalar queue serves tiles in the order the PE consumes them ----
        xts = []
        x1_pending = None
        for ct in cts:
            x_t = xpool.tile([P, KD, CT], BF16, name=f"xt_{ct}", tag="xt")
            if ct == 0:
                g0 = 1 if micro else 2
                for q in range(KD // g0):
                    dma(nc.scalar, x_t[:, g0 * q:g0 * q + g0, :],
                        xt[ct][:, g0 * q:g0 * q + g0, :])
            elif ct == 1:
                x1_pending = (x_t, ct)
            xts.append(x_t)
        hs = [
            [hpool.tile([P, CT], BF16, name=f"h_{ct}_{ht}", tag="h")
             for ht in range(KH)]
            for ct in cts
        ]

        # ---- stage A ----
        for ht4 in range(NH4):
            gt = wpool.tile([P, KD, 512], BF16, name=f"g_{ht4}", tag="w")
            if ht4 == 0:
                g0 = 1 if micro else 2
                for q in range(KD // g0):
                    dma(nc.sync, gt[:, g0 * q:g0 * q + g0, :],
                        gw[ht4][:, g0 * q:g0 * q + g0, :])
            else:
                dma(nc.sync, gt[:, :, :], gw[ht4][:, :, :])
            ut = wpool.tile([P, KD, 512], BF16, name=f"u_{ht4}", tag="w")
            if ht4 == 0 and x1_pending is not None:
                x1_t, x1_ct = x1_pending
                dma(nc.scalar, ut[:, 0:4, :], uw[ht4][:, 0:4, :])
                dma(nc.scalar, x1_t[:, :, :], xt[x1_ct][:, :, :])
                dma(nc.scalar, ut[:, 4:8, :], uw[ht4][:, 4:8, :])
                for ct in cts[2:]:
                    dma(nc.scalar, xts[ct][:, :, :], xt[ct][:, :, :])
            else:
                dma(nc.scalar, ut[:, :, :], uw[ht4][:, :, :])
            for ci in range(len(cts)):
                tmps = []
                for sub in range(4):
                    ht = ht4 * 4 + sub
                    pg = pspool.tile([P, CT], F32, name=f"pg_{ht}_{ci}", tag="ps")
                    for kt in range(KD):
                        nc.tensor.matmul(
                            pg[:, :],
                            gt[:, kt, sub * P:(sub + 1) * P],
                            xts[ci][:, kt, :],
                            start=(kt == 0),
                            stop=(kt == KD - 1),
                        )
                    tmp = tpool.tile([P, CT], F32, name=f"t_{ht}_{ci}", tag="t")
                    nc.scalar.activation(tmp[:, :], pg[:, :], AF.Silu)
                    tmps.append(tmp)
                for sub in range(4):
                    ht = ht4 * 4 + sub
                    pu = pspool.tile([P, CT], F32, name=f"pu_{ht}_{ci}", tag="ps")
                    for kt in range(KD):
                        nc.tensor.matmul(
                            pu[:, :],
                            ut[:, kt, sub * P:(sub + 1) * P],
                            xts[ci][:, kt, :],
                            start=(kt == 0),
                            stop=(kt == KD - 1),
                        )
                    nc.vector.tensor_mul(
                        hs[ci][ht][:, :], tmps[sub][:, :], pu[:, :]
                    )

        # ---- stage B: dw SBUF-resident, 4 phases of (dc, ci-pair) ----
        dts = {}
        for dc in range(2):
            for hf in range(4):
                dt_ = dpool.tile([P, KH // 4, 512], BF16,
                                 name=f"d_{dc}_{hf}", tag="dw")
                dma(nc.sync, dt_[:, :, :],
                    dw[dc, hf // 2][:, (hf % 2) * 4:(hf % 2) * 4 + 4, :])
                dts[(dc, hf)] = dt_

        for dc in range(2):
            for cis in ((0, 1), (2, 3)):
                pos = {}
                for ci in cis:
                    for m in range(CT // P):
                        pos[(ci, m)] = pspool.tile(
                            [P, 512], F32, name=f"po_{dc}_{ci}_{m}", tag="ps"
                        )
                ots = {
                    ci: opool.tile([P, CT // P, 512], BF16,
                                   name=f"o_{dc}_{ci}", tag="o")
                    for ci in cis
                }
                for hf in range(4):
                    dt_ = dts[(dc, hf)]
                    for kb in range(KH // 4):
                        kh = hf * (KH // 4) + kb
                        for ci in cis:
                            for m in range(CT // P):
                                nc.tensor.matmul(
                                    pos[(ci, m)][:, :],
                                    hs[ci][kh][:, m * P:(m + 1) * P],
                                    dt_[:, kb, :],
                                    start=(kh == 0),
                                    stop=(kh == KH - 1),
                                )
                for ci in cis:
                    ct = ci
                    for m in range(CT // P):
                        j = ct * (CT // P) + m
                        if m % 2 == 0:
                            nc.scalar.mul(
                                ots[ci][:, m, :], pos[(ci, m)][:, :],
                                p_sb[:, j:j + 1]
                            )
                        else:
                            nc.vector.tensor_scalar_mul(
                                ots[ci][:, m, :], pos[(ci, m)][:, :],
                                p_sb[:, j:j + 1]
                            )
                        last_round = dc == 1 and cis == (2, 3)
                        if not dma_out:
                            pass
                        elif store_mode == "gpsimd_half":
                            if last_round:
                                nc.gpsimd.dma_start(
                                    y[ct, dc][:, m:m + 1, :],
                                    ots[ci][:, m:m + 1, :],
                                )
                            elif m % 2 == 1:
                                nc.gpsimd.dma_start(
                                    y[ct, dc][:, m - 1:m + 1, :],
                                    ots[ci][:, m - 1:m + 1, :],
                                )
                        elif last_round and m % 2 == 1:
                            eng = nc.sync if m == 1 else nc.scalar
                            eng.dma_start(
                                y[ct, dc][:, m - 1:m + 1, :],
                                ots[ci][:, m - 1:m + 1, :],
                            )
                        elif m == CT // P - 1:
                            eng = nc.sync if dc == 0 else nc.scalar
                            eng.dma_start(y[ct, dc][:, :, :], ots[ci][:, :, :])


def emit_expert_ffn_v3(tc, xt, gw, uw, dw, pv, y, dma_in=True, dma_out=True,
                       micro=True, store_mode="gpsimd_half", full_reuse=False):
    """Baseline body + weight-tile reuse: superblock 1 iterates ht4 groups
    and dc halves in REVERSE and reuses the gate/up tiles of ht4 {3,2} and
    the down tiles of dc=1 still resident in SBUF from superblock 0
    (wpool/dpool bufs 3->4), skipping 6MB of the 24MB per-rep weight DMA.
    Superblock 1's stage-B kh accumulation runs 15..0 so h tiles are
    consumed in the order stage A produces them."""
    nc = tc.nc

    def dma(engine, dst, src):
        if dma_in:
            engine.dma_start(dst, src)
        else:
            ix = (slice(None),) + (slice(0, 1),) * (len(dst.shape) - 1)
            engine.dma_start(dst[ix], src[ix])

    sbs = [list(range(s, min(s + 2, NCT))) for s in range(0, NCT, 2)]
    w_cache = {}
    d_cache = {}

    with (
        tc.tile_pool(name="xpool", bufs=2) as xpool,
        tc.tile_pool(name="wpool", bufs=8 if full_reuse else 4) as wpool,
        tc.tile_pool(name="hpool", bufs=36) as hpool,
        tc.tile_pool(name="dpool", bufs=8 if full_reuse else 4) as dpool,
        tc.tile_pool(name="tpool", bufs=5) as tpool,
        tc.tile_pool(name="opool", bufs=2) as opool,
        tc.tile_pool(name="ppool", bufs=1) as ppool,
        tc.tile_pool(name="pspool", bufs=8, space="PSUM") as pspool,
    ):
        p_sb = ppool.tile([P, C // P], F32)
        dma(nc.scalar, p_sb[:, :], pv[:, :])

        for sbi, cts in enumerate(sbs):
            first_sb = sbi == 0
            if full_reuse:
                # all weights SBUF-resident after sb0: forward order both
                # superblocks, sb1 issues zero weight DMAs
                ht4_order = list(range(NH4))
                dc_order = [0, 1]
                reuse_w = set() if first_sb else {0, 1, 2, 3}
                reuse_d = set() if first_sb else {0, 1}
            else:
                ht4_order = (list(range(NH4)) if first_sb
                             else list(range(NH4))[::-1])
                dc_order = [0, 1] if first_sb else [1, 0]
                reuse_w = set() if first_sb else {3, 2}
                reuse_d = set() if first_sb else {1}

            xts = []
            x1_pending = None
            for ct in cts:
                x_t = xpool.tile([P, KD, CT], BF16, name=f"xt_{ct}", tag="xt")
                if ct == 0:
                    g0 = 1 if micro else 2
                    for q in range(KD // g0):
                        dma(nc.scalar, x_t[:, g0 * q:g0 * q + g0, :],
                            xt[ct][:, g0 * q:g0 * q + g0, :])
                elif first_sb:
                    x1_pending = (x_t, ct)
                else:
                    dma(nc.scalar, x_t[:, :, :], xt[ct][:, :, :])
                xts.append(x_t)
            hs = [
                [hpool.tile([P, CT], BF16, name=f"h_{ct}_{ht}", tag="h")
                 for ht in range(KH)]
                for ct in cts
            ]

            # ---- stage A ----
            for ht4 in ht4_order:
                if ht4 in reuse_w:
                    gt, ut = w_cache[ht4]
                else:
                    gt = wpool.tile([P, KD, 512], BF16,
                                    name=f"g_{sbi}_{ht4}", tag="w")
                    if ht4 == 0 and first_sb:
                        g0 = 1 if micro else 2
                        for q in range(KD // g0):
                            dma(nc.sync, gt[:, g0 * q:g0 * q + g0, :],
                                gw[ht4][:, g0 * q:g0 * q + g0, :])
                    else:
                        dma(nc.sync, gt[:, :, :], gw[ht4][:, :, :])
                    ut = wpool.tile([P, KD, 512], BF16,
                                    name=f"u_{sbi}_{ht4}", tag="w")
                    if ht4 == 0 and x1_pending is not None:
                        x1_t, x1_ct = x1_pending
                        dma(nc.scalar, ut[:, 0:4, :], uw[ht4][:, 0:4, :])
                        dma(nc.scalar, x1_t[:, :, :], xt[x1_ct][:, :, :])
                        dma(nc.scalar, ut[:, 4:8, :], uw[ht4][:, 4:8, :])
                    else:
                        dma(nc.scalar, ut[:, :, :], uw[ht4][:, :, :])
                    if first_sb:
                        w_cache[ht4] = (gt, ut)
                for ci in range(len(cts)):
                    tmps = []
                    for sub in range(4):
                        ht = ht4 * 4 + sub
                        pg = pspool.tile([P, CT], F32,
                                         name=f"pg_{sbi}_{ht}_{ci}", tag="ps")
                        for kt in range(KD):
                            nc.tensor.matmul(
                                pg[:, :],
                                gt[:, kt, sub * P:(sub + 1) * P],
                                xts[ci][:, kt, :],
                                start=(kt == 0),
                                stop=(kt == KD - 1),
                            )
                        tmp = tpool.tile([P, CT], F32,
                                         name=f"t_{sbi}_{ht}_{ci}", tag="t")
                        nc.scalar.activation(tmp[:, :], pg[:, :], AF.Silu)
                        tmps.append(tmp)
                    for sub in range(4):
                        ht = ht4 * 4 + sub
                        pu = pspool.tile([P, CT], F32,
                                         name=f"pu_{sbi}_{ht}_{ci}", tag="ps")
                        for kt in range(KD):
                            nc.tensor.matmul(
                                pu[:, :],
                                ut[:, kt, sub * P:(sub + 1) * P],
                                xts[ci][:, kt, :],
                                start=(kt == 0),
                                stop=(kt == KD - 1),
                            )
                        nc.vector.tensor_mul(
                            hs[ci][ht][:, :], tmps[sub][:, :], pu[:, :]
                        )

            # ---- stage B ----
            for dci, dc in enumerate(dc_order):
                pos = {}
                for ci in range(len(cts)):
                    for m in range(CT // P):
                        pos[(ci, m)] = pspool.tile(
                            [P, 512], F32, name=f"po_{sbi}_{dc}_{ci}_{m}",
                            tag="ps"
                        )
                ots = [
                    opool.tile([P, CT // P, 512], BF16,
                               name=f"o_{sbi}_{dc}_{ci}", tag="o")
                    for ci in range(len(cts))
                ]
                fwd = first_sb or full_reuse
                hf_order = list(range(4)) if fwd else list(range(4))[::-1]
                kb_order = (list(range(KH // 4)) if fwd
                            else list(range(KH // 4))[::-1])
                kh_first = 0 if fwd else KH - 1
                kh_last = KH - 1 if fwd else 0
                for hf in hf_order:
                    if dc in reuse_d:
                        dt_ = d_cache[(dc, hf)]
                    elif (dc, hf, sbi) in d_cache:
                        dt_ = d_cache[(dc, hf, sbi)]
                    else:
                        dt_ = dpool.tile([P, KH // 4, 512], BF16,
                                         name=f"d_{sbi}_{dc}_{hf}", tag="dw")
                        dma(nc.sync, dt_[:, :, :],
                            dw[dc, hf // 2][:, (hf % 2) * 4:(hf % 2) * 4 + 4, :])
                        if first_sb:
                            d_cache[(dc, hf)] = dt_
                        else:
                            d_cache[(dc, hf, sbi)] = dt_
                    for kb in kb_order:
                        kh = hf * (KH // 4) + kb
                        for ci in range(len(cts)):
                            for m in range(CT // P):
                                nc.tensor.matmul(
                                    pos[(ci, m)][:, :],
                                    hs[ci][kh][:, m * P:(m + 1) * P],
                                    dt_[:, kb, :],
                                    start=(kh == kh_first),
                                    stop=(kh == kh_last),
                                )
                for ci, ct in enumerate(cts):
                    for m in range(CT // P):
                        j = ct * (CT // P) + m
                        if m % 2 == 0:
                            nc.scalar.mul(
                                ots[ci][:, m, :], pos[(ci, m)][:, :],
                                p_sb[:, j:j + 1]
                            )
                        else:
                            nc.vector.tensor_scalar_mul(
                                ots[ci][:, m, :], pos[(ci, m)][:, :],
                                p_sb[:, j:j + 1]
                            )
                        last_round = (micro and sbi == len(sbs) - 1
                                      and dci == 1)
                        if not dma_out:
                            pass
                        elif last_round:
                            nc.gpsimd.dma_start(
                                y[ct, dc][:, m:m + 1, :],
                                ots[ci][:, m:m + 1, :],
                            )
                        elif m % 2 == 1:
                            nc.gpsimd.dma_start(
                                y[ct, dc][:, m - 1:m + 1, :],
                                ots[ci][:, m - 1:m + 1, :],
                            )


def build_nc(reps_loop=False, max_reps=512, dma_in=True, dma_out=True, unroll=1,
             stagger=False, micro=True, store_mode="gpsimd_half", bodies=1,
             v2=False, v3=True, full_reuse=False):
    """Build the per-core Bass program. With reps_loop, the whole body runs
    inside a For_i whose trip count is read from an int32 input "reps"."""
    nc = bacc.Bacc(None, target_bir_lowering=False)

    def emit(tc, xt, gw, uw, dw, pv, y, dma_in=True, dma_out=True, micro=True):
        if v3:
            return emit_expert_ffn_v3(tc, xt, gw, uw, dw, pv, y, dma_in=dma_in,
                                      dma_out=dma_out, micro=micro,
                                      store_mode=store_mode,
                                      full_reuse=full_reuse)
        fn = emit_expert_ffn_v2 if v2 else emit_expert_ffn
        return fn(tc, xt, gw, uw, dw, pv, y, dma_in=dma_in,
                  dma_out=dma_out, micro=micro, store_mode=store_mode)
    with tile.TileContext(nc) as tc:
        xt = nc.dram_tensor("xt", [NCT, P, KD, CT], BF16, kind="ExternalInput")
        gw = nc.dram_tensor("gw", [NH4, P, KD, 512], BF16, kind="ExternalInput")
        uw = nc.dram_tensor("uw", [NH4, P, KD, 512], BF16, kind="ExternalInput")
        pv = nc.dram_tensor("pv", [P, C // P], F32, kind="ExternalInput")
        dw = nc.dram_tensor("dw", [2, 2, P, KH // 2, 512], BF16, kind="ExternalInput")
        y = nc.dram_tensor("y", [NCT, 2, P, CT // P, 512], BF16,
                           kind="ExternalOutput")
        if reps_loop:
            reps = nc.dram_tensor("reps", [1, 1], mybir.dt.int32, kind="ExternalInput")
            with tc.tile_pool(name="rpool", bufs=1) as rpool:
                r_sb = rpool.tile([1, 1], mybir.dt.int32)
                nc.sync.dma_start(r_sb[:, :], reps[:, :])
                rv = nc.values_load(
                    r_sb[0:1, 0:1],
                    min_val=0,
                    max_val=max_reps,
                    skip_runtime_bounds_check=True,
                )
            with tc.For_i(0, rv, 1, staggered_reset=stagger):
                for _ in range(unroll):
                    emit(tc, xt, gw, uw, dw, pv, y,
                         dma_in=dma_in, dma_out=dma_out, micro=micro)
        else:
            for _ in range(bodies):
                emit(tc, xt, gw, uw, dw, pv, y, dma_in=dma_in, dma_out=dma_out,
                     micro=micro)
    nc.compile()
    return nc


def pack_inputs(x_pad, gate_w_e, up_w_e, down_w_e, p_pad):
    """Pack one expert's inputs into the SBUF-tile-order DRAM layouts."""
    # xt [NCT, 128, KD, 512]: [ct, p, kt, tok] = x_pad[ct*512+tok, kt*128+p]
    xt = np.ascontiguousarray(
        x_pad.reshape(NCT, CT, KD, P).transpose(0, 3, 2, 1).astype(NPBF16)
    )
    # gw/uw [NH4, 128, KD, 512]: [b, p, kt, h] = w[b*512+h, kt*128+p]
    gw = np.ascontiguousarray(
        gate_w_e.reshape(NH4, 512, KD, P).transpose(0, 3, 2, 1).astype(NPBF16)
    )
    uw = np.ascontiguousarray(
        up_w_e.reshape(NH4, 512, KD, P).transpose(0, 3, 2, 1).astype(NPBF16)
    )
    # dw [2, 2, 128, KH//2, 512]: [dc, hf, p, kb, d] = down[dc*512+d, hf*1024+kb*128+p]
    dw = np.ascontiguousarray(
        down_w_e.reshape(2, 512, 2, KH // 2, P).transpose(0, 2, 4, 3, 1).astype(NPBF16)
    )
    pv = np.ascontiguousarray(p_pad.reshape(C // P, P).T)
    return {"xt": xt, "gw": gw, "uw": uw, "dw": dw, "pv": pv}


def unpack_y(y_pack):
    """y_pack [NCT, 2, 128, 4, 512] bf16 -> y [C, D] f32."""
    return np.ascontiguousarray(
        y_pack.transpose(0, 3, 2, 1, 4).reshape(C, D).astype(np.float32)
    )


def route_and_dispatch(x, router_w):
    """Host router + top-2 dispatch (matches softmax/top_k/renorm of the
    reference exactly)."""
    logits = x @ router_w.T                      # [T, E]
    t_ar = np.arange(T)
    i1 = np.argmax(logits, axis=1)
    l1 = logits[t_ar, i1]
    lm = logits.copy()
    lm[t_ar, i1] = -np.inf
    i2 = np.argmax(lm, axis=1)
    l2 = lm[t_ar, i2]
    e2 = np.exp(l2 - l1)
    p1 = 1.0 / (1.0 + e2)
    p2 = e2 / (1.0 + e2)

    ee = np.concatenate([i1, i2])                # [2T] expert of each pair
    tt = np.concatenate([t_ar, t_ar])            # [2T] token of each pair
    pp = np.concatenate([p1, p2]).astype(np.float32)
    counts = np.bincount(ee, minlength=E)
    starts = np.zeros(E, np.int64)
    starts[1:] = np.cumsum(counts)[:-1]
    order = np.argsort(ee, kind="stable")
    pos = np.empty(2 * T, np.int64)
    pos[order] = np.arange(2 * T) - starts[ee[order]]
    return ee, tt, pp, pos, counts, starts, order


def kernel(**inputs):
    x = np.ascontiguousarray(
        np.asarray(inputs["hidden_states"], np.float32).reshape(T, D)
    )
    router_w = np.asarray(inputs["router_w"], np.float32)
    gate_w = np.asarray(inputs["gate_w"], np.float32)
    up_w = np.asarray(inputs["up_w"], np.float32)
    down_w = np.asarray(inputs["down_w"], np.float32)

    ee, tt, pp, pos, counts, starts, order = route_and_dispatch(x, router_w)

    in_maps = []
    for e in range(E):
        n_e = min(int(counts[e]), C)
        sel = order[starts[e]:starts[e] + n_e]   # pairs dispatched to core e
        xp = np.zeros((C, D), np.float32)
        xp[:n_e] = x[tt[sel]]
        pvec = np.zeros(C, np.float32)
        pvec[:n_e] = pp[sel]
        in_maps.append(pack_inputs(xp, gate_w[e], up_w[e], down_w[e], pvec))

    nc = build_nc()
    res = run_bass_kernel_spmd(nc, in_maps, core_ids=list(range(E)))
    ys = np.stack(
        [unpack_y(res.results[e]["y"]) for e in range(E)]
    ).reshape(E * C, D)

    ok = pos < C
    contrib = np.zeros((2 * T, D), np.float32)
    g = ee * C + pos
    contrib[ok] = ys[g[ok]]
    # capacity-overflow fallback: exact fp32 host compute for the few pairs
    # beyond capacity (~0.8% of pairs for the seed-0 routing), batched per
    # expert
    if not ok.all():
        bad = np.nonzero(~ok)[0]
        for e in np.unique(ee[bad]):
            js = bad[ee[bad] == e]
            xb = x[tt[js]]
            gb = xb @ gate_w[e].T
            ub = xb @ up_w[e].T
            hb = (gb / (1.0 + np.exp(-gb))) * ub
            contrib[js] = (hb @ down_w[e].T) * pp[js, None]
    out = contrib[:T] + contrib[T:]
    return out.reshape(B, S, D).astype(np.float32)



# revision 31
# speedup vs baseline: 1.0293x; 1.0293x over previous
"""MoE layer (top-2 of 8 experts) for 8 Trainium2 NeuronCores.

Strategy: expert-parallel. Host computes the (tiny) router + top-2 dispatch in
numpy; core e runs expert e's FFN over its dispatched tokens (padded to a fixed
capacity C); host combines the two expert outputs per token.

All device matmuls are [K=128]x[M=128]x[N=512] bf16 (1 cycle/row, fp32 PSUM).
bf16 beats fp32r here: same 1 cyc/row stream rate, but fast-weight-load keeps
the per-matmul 128-cycle LDWEIGHTS hidden (fp32r has no FWL and measures ~25%
slower end-to-end), and input DMA bytes halve. max-rel error ~4e-3 vs the
fp32 reference (gate 2e-2).
  gate^T/up^T [H, Ct] = gwT/uwT.T @ xt   (contraction over D, 8 k-tiles)
  h = silu(gate) * up                    (SBUF-resident [128, 512] bf16 tiles)
  y [Ct, D] = (h.T @ dwT) * p            (contraction over H, 16 k-tiles,
                                          combine-prob scale fused in eviction)

Host pre-packs weights/tokens into SBUF-tile order so every load is one large
DMA with >=1KB contiguous lines. Queue layout: weights on the two HWDGE queues
(gate+down on sync, up+x+p on scalar), y stores alone on the gpsimd/SWDGE
queue so they never delay the next superblock's x loads. Stage-B evictions
alternate scalar/DVE and stores go out in half-batches so the end-of-body
DMA-quiesce tail stays short. y is written bf16 (halves the tail transfer).

Second session: the default body is now emit_expert_ffn_v3 (reuse_n=5) —
identical instruction stream to the original, but superblock 1 iterates
ht4/dc/kh in REVERSE and reuses ALL gate/up/down weight tiles still
resident in SBUF from superblock 0 (wpool bufs=8, dpool bufs=8): every
weight is read from HBM exactly once per rep, cutting per-rep DMA
32->20MB. The win is power/thermal — fewer HBM bytes at identical compute
in the throttled regime. Paired ABBA ladder (graded LO2/HI34 protocol,
median per step): reuse {3,2}+{dc1} -11us vs original; reuse {3,2,1}
-9us more; reuse all gate/up -10us more; +full dw reuse ~tie (-5us
median, shipped for minimum bytes). CRITICAL: the reversal is load-
bearing — an earlier full-reuse variant with FORWARD iteration order both
superblocks measured tie-or-WORSE than partial reuse, as did a
single-superblock rewrite (v2); keep the reversed structure.

Optimization notes (second session): this body is at the bf16 PE roofline.
1536 N=512 matmuls = 786k PE cycles = 327.7us @ 2.4GHz; the measured For_i
marginal per-rep at short bursts is 328.1us (reps pipeline across the loop
back-edge; TimelineSim agrees: 328.7us marginal, PE 92.7% busy, LDWEIGHTS
fully hidden). The reported ~380-400us/rep number is power-bound, not
structure-bound: sustained bursts throttle the PE to ~1.9GHz (P0), where the
roofline is ~410us; the LO=2/HI=34 protocol straddles the transition and
run-to-run thermal state dominates (+-15us for identical binaries). Variants
tried and rejected as no-better-or-worse under paired measurement: HWDGE
full-tile stores (sync/scalar rings), staggered_reset For_i, a
single-superblock body that halves gate/up weight DMA (16->8MB/rep; equal
cold, reproducibly ~8-13us WORSE in the deep-hot regime), fp8 matmuls
(max-rel 6.6e-2 >> 2e-2 budget; DoubleRow also loses FWL below FD>=256),
and bf16-PSUM N=1024 matmuls (TRN2 requires fp32 matmul output; TRN3-only).
"""

import ml_dtypes
import numpy as np

import concourse.bass as bass
import concourse.mybir as mybir
import concourse.tile as tile
from concourse import bacc
from concourse.bass_utils import run_bass_kernel_spmd

E = 8
TOP_K = 2
B, S, D, H = 4, 2048, 1024, 2048
T = B * S
C = 2048          # per-expert token capacity; overflow pairs (seed-0: ~137
                  # of 16384, counts max 2175) fall back to exact host compute
CT = 512          # token tile
P = 128
NCT = C // CT     # 4
KD = D // P       # 8  k-tiles for gate/up
KH = H // P       # 16 k-tiles for down
NH4 = H // 512    # 4  groups of 4 h-blocks
F32 = mybir.dt.float32
F32R = mybir.dt.float32r
BF16 = mybir.dt.bfloat16
NPBF16 = ml_dtypes.bfloat16
AF = mybir.ActivationFunctionType


def emit_expert_ffn(tc, xt, gw, uw, dw, pv, y, dma_in=True, dma_out=True,
                    micro=True, store_mode="gpsimd_half"):
    """Emit one expert's FFN.

    DRAM tensors (all pre-packed on host):
      xt [NCT, 128, KD, 512] bf16 - tokens, transposed per ct tile
      gw/uw [NH4, 128, KD, 512] bf16 - gate/up weights per 4-h-block group
      dw [2, 2, 128, KH//2, 512] bf16 - down weights per (dc, kh-half)
      pv [128, C//128] f32 - combine probs (token-partition layout)
      y  [NCT, 2, 128, 4, 512] bf16 out - [ct, dc, p, m, 512]

    dma_in/dma_out: timing-experiment knobs that drop the input/output DMAs
    (compute runs on whatever is in SBUF); both True for the real kernel.
    """
    nc = tc.nc

    def dma(engine, dst, src):
        if dma_in:
            engine.dma_start(dst, src)
        else:
            # timing experiment: 1-element-per-partition DMA keeps the tile
            # written (scheduler requirement) with negligible HBM traffic
            ix = (slice(None),) + (slice(0, 1),) * (len(dst.shape) - 1)
            engine.dma_start(dst[ix], src[ix])
    # superblocks of up to 2 token tiles sharing one weight pass
    sbs = [list(range(s, min(s + 2, NCT))) for s in range(0, NCT, 2)]

    with (
        tc.tile_pool(name="xpool", bufs=2) as xpool,
        tc.tile_pool(name="wpool", bufs=3) as wpool,
        tc.tile_pool(name="hpool", bufs=36) as hpool,
        tc.tile_pool(name="dpool", bufs=3) as dpool,
        tc.tile_pool(name="tpool", bufs=5) as tpool,
        tc.tile_pool(name="opool", bufs=2) as opool,
        tc.tile_pool(name="ppool", bufs=1) as ppool,
        tc.tile_pool(name="pspool", bufs=8, space="PSUM") as pspool,
    ):
        p_sb = ppool.tile([P, C // P], F32)
        dma(nc.scalar, p_sb[:, :], pv[:, :])

        for cts in sbs:
            # ---- token tiles ----
            # on the scalar (HWDGE) queue: the gpsimd/SWDGE queue carries the
            # y stores, which would delay the next superblock's x loads by
            # ~10us of descriptor-gen right when the PE needs them
            first_sb = cts[0] == 0
            xts = []
            x1_pending = None
            for ct in cts:
                x_t = xpool.tile([P, KD, CT], BF16, name=f"xt_{ct}", tag="xt")
                if ct == 0:
                    # k-slice granularity on the first tile: the first matmul
                    # group consumes k-slices in order, so it can start after
                    # slice 0 instead of after the full half-tile
                    g0 = 1 if micro else 2
                    for q in range(KD // g0):
                        dma(nc.scalar, x_t[:, g0 * q:g0 * q + g0, :],
                            xt[ct][:, g0 * q:g0 * q + g0, :])
                elif first_sb:
                    # defer x1's load into the ht4==0 block, between the two
                    # halves of the up-weight load (both on the scalar queue):
                    # u0's first half then arrives before the first up-matmul
                    # group needs it, x1 before the ci=1 gate group
                    x1_pending = (x_t, ct)
                else:
                    dma(nc.scalar, x_t[:, :, :], xt[ct][:, :, :])
                xts.append(x_t)
            hs = [
                [
                    hpool.tile([P, CT], BF16, name=f"h_{ct}_{ht}", tag="h")
                    for ht in range(KH)
                ]
                for ct in cts
            ]

            # ---- stage A: gate/up matmuls + silu*mul -> h ----
            for ht4 in range(NH4):
                gt = wpool.tile([P, KD, 512], BF16, name=f"g_{ht4}", tag="w")
                if ht4 == 0 and first_sb:
                    # quarter-granularity on the very first load so the first
                    # matmuls start ~2us earlier out of the cold start
                    g0 = 1 if micro else 2
                    for q in range(KD // g0):
                        dma(nc.sync,
                            gt[:, g0 * q:g0 * q + g0, :],
                            gw[ht4][:, g0 * q:g0 * q + g0, :]
                        )
                else:
                    dma(nc.sync, gt[:, :, :], gw[ht4][:, :, :])
                ut = wpool.tile([P, KD, 512], BF16, name=f"u_{ht4}", tag="w")
                if ht4 == 0 and x1_pending is not None:
                    x1_t, x1_ct = x1_pending
                    dma(nc.scalar, ut[:, 0:4, :], uw[ht4][:, 0:4, :])
                    dma(nc.scalar, x1_t[:, :, :], xt[x1_ct][:, :, :])
                    dma(nc.scalar, ut[:, 4:8, :], uw[ht4][:, 4:8, :])
                else:
                    dma(nc.scalar, ut[:, :, :], uw[ht4][:, :, :])
                # ct-major, all-gate-then-all-up: gt's last use lands at ~75%
                # of the group so the next group's weight DMA overlaps compute
                for ci in range(len(cts)):
                    tmps = []
                    for sub in range(4):
                        ht = ht4 * 4 + sub
                        pg = pspool.tile([P, CT], F32, name=f"pg_{ht}_{ci}", tag="ps")
                        for kt in range(KD):
                            nc.tensor.matmul(
                                pg[:, :],
                                gt[:, kt, sub * P:(sub + 1) * P],
                                xts[ci][:, kt, :],
                                start=(kt == 0),
                                stop=(kt == KD - 1),
                            )
                        tmp = tpool.tile([P, CT], F32, name=f"t_{ht}_{ci}", tag="t")
                        nc.scalar.activation(tmp[:, :], pg[:, :], AF.Silu)
                        tmps.append(tmp)
                    for sub in range(4):
                        ht = ht4 * 4 + sub
                        pu = pspool.tile([P, CT], F32, name=f"pu_{ht}_{ci}", tag="ps")
                        for kt in range(KD):
                            nc.tensor.matmul(
                                pu[:, :],
                                ut[:, kt, sub * P:(sub + 1) * P],
                                xts[ci][:, kt, :],
                                start=(kt == 0),
                                stop=(kt == KD - 1),
                            )
                        nc.vector.tensor_mul(
                            hs[ci][ht][:, :], tmps[sub][:, :], pu[:, :]
                        )

            # ---- stage B: down matmuls + prob scale -> y ----
            for dc in range(2):
                pos = {}
                for ci in range(len(cts)):
                    for m in range(CT // P):
                        pos[(ci, m)] = pspool.tile(
                            [P, 512], F32, name=f"po_{dc}_{ci}_{m}", tag="ps"
                        )
                ots = [
                    opool.tile([P, CT // P, 512], BF16,
                               name=f"o_{dc}_{ci}", tag="o")
                    for ci in range(len(cts))
                ]
                for hf in range(4):
                    dt_ = dpool.tile([P, KH // 4, 512], BF16, name=f"d_{dc}_{hf}", tag="dw")
                    dma(nc.sync,
                        dt_[:, :, :], dw[dc, hf // 2][:, (hf % 2) * 4:(hf % 2) * 4 + 4, :]
                    )
                    for kb in range(KH // 4):
                        kh = hf * (KH // 4) + kb
                        for ci in range(len(cts)):
                            for m in range(CT // P):
                                nc.tensor.matmul(
                                    pos[(ci, m)][:, :],
                                    hs[ci][kh][:, m * P:(m + 1) * P],
                                    dt_[:, kb, :],
                                    start=(kh == 0),
                                    stop=(kh == KH - 1),
                                )
                for ci, ct in enumerate(cts):
                    for m in range(CT // P):
                        j = ct * (CT // P) + m
                        # split evictions across scalar/DVE so the post-last-
                        # matmul drain chain is half as long
                        if m % 2 == 0:
                            nc.scalar.mul(
                                ots[ci][:, m, :], pos[(ci, m)][:, :], p_sb[:, j:j + 1]
                            )
                        else:
                            nc.vector.tensor_scalar_mul(
                                ots[ci][:, m, :], pos[(ci, m)][:, :], p_sb[:, j:j + 1]
                            )
                        # store policy; see store_mode in build_nc
                        last_round = micro and dc == 1 and cts[-1] == NCT - 1
                        if not dma_out:
                            pass
                        elif store_mode == "gpsimd_half":
                            # half-batched SWDGE stores (16/body)
                            if last_round:
                                nc.gpsimd.dma_start(
                                    y[ct, dc][:, m:m + 1, :],
                                    ots[ci][:, m:m + 1, :],
                                )
                            elif m % 2 == 1:
                                nc.gpsimd.dma_start(
                                    y[ct, dc][:, m - 1:m + 1, :],
                                    ots[ci][:, m - 1:m + 1, :],
                                )
                        elif store_mode in ("sync_full", "split_full"):
                            # one 512KB HWDGE store per (ct, dc) after all 4
                            # evictions; last round splits across both HWDGE
                            # rings so the end-of-body tail is half as long
                            if last_round and m % 2 == 1:
                                eng = nc.sync if m == 1 else nc.scalar
                                eng.dma_start(
                                    y[ct, dc][:, m - 1:m + 1, :],
                                    ots[ci][:, m - 1:m + 1, :],
                                )
                            elif m == CT // P - 1:
                                eng = (
                                    nc.sync
                                    if (store_mode == "sync_full" or dc == 0)
                                    else nc.scalar
                                )
                                eng.dma_start(
                                    y[ct, dc][:, :, :], ots[ci][:, :, :]
                                )
                        elif store_mode == "sync_half":
                            if last_round:
                                eng = nc.sync if m % 2 == 0 else nc.scalar
                                eng.dma_start(
                                    y[ct, dc][:, m:m + 1, :],
                                    ots[ci][:, m:m + 1, :],
                                )
                            elif m % 2 == 1:
                                nc.sync.dma_start(
                                    y[ct, dc][:, m - 1:m + 1, :],
                                    ots[ci][:, m - 1:m + 1, :],
                                )
                        else:
                            raise ValueError(store_mode)


def emit_expert_ffn_v2(tc, xt, gw, uw, dw, pv, y, dma_in=True, dma_out=True,
                       micro=True, store_mode="split_full"):
    """Single-superblock body: all 4 token tiles share one weight pass, so
    gate/up weights stream from HBM once per rep (8MB) instead of twice
    (16MB), and down weights are SBUF-resident for the whole stage B.

    Stage A: per ht4 group (4 h-blocks of 128), for each of 4 token tiles:
      gate psum x4 -> silu -> tmp; up psum x4 -> h = tmp*up.
    Stage B: dw fully loaded up front; for dc in 2, for ci-pair in
      [(0,1),(2,3)]: 8 psum banks accumulate over all 16 kh tiles, evict
      with the combine-prob scale fused, store per (ct, dc) as one 512KB
      HWDGE transfer (dc0 on sync ring, dc1 on scalar ring).
    """
    nc = tc.nc

    def dma(engine, dst, src):
        if dma_in:
            engine.dma_start(dst, src)
        else:
            ix = (slice(None),) + (slice(0, 1),) * (len(dst.shape) - 1)
            engine.dma_start(dst[ix], src[ix])

    cts = list(range(NCT))
    with (
        tc.tile_pool(name="xpool", bufs=4) as xpool,
        tc.tile_pool(name="wpool", bufs=3) as wpool,
        tc.tile_pool(name="hpool", bufs=64) as hpool,
        tc.tile_pool(name="dpool", bufs=8) as dpool,
        tc.tile_pool(name="tpool", bufs=5) as tpool,
        tc.tile_pool(name="opool", bufs=4) as opool,
        tc.tile_pool(name="ppool", bufs=1) as ppool,
        tc.tile_pool(name="pspool", bufs=8, space="PSUM") as pspool,
    ):
        p_sb = ppool.tile([P, C // P], F32)
        dma(nc.scalar, p_sb[:, :], pv[:, :])

        # ---- token tiles: x0 micro-sliced for fast cold start; x1 deferred
        # between the two halves of u0's load; x2/x3 deferred after u0 so
        # the scalar queue serves tiles in the order the PE consumes them ----
        xts = []
        x1_pending = None
        for ct in cts:
            x_t = xpool.tile([P, KD, CT], BF16, name=f"xt_{ct}", tag="xt")
            if ct == 0:
                g0 = 1 if micro else 2
                for q in range(KD // g0):
                    dma(nc.scalar, x_t[:, g0 * q:g0 * q + g0, :],
                        xt[ct][:, g0 * q:g0 * q + g0, :])
            elif ct == 1:
                x1_pending = (x_t, ct)
            xts.append(x_t)
        hs = [
            [hpool.tile([P, CT], BF16, name=f"h_{ct}_{ht}", tag="h")
             for ht in range(KH)]
            for ct in cts
        ]

        # ---- stage A ----
        for ht4 in range(NH4):
            gt = wpool.tile([P, KD, 512], BF16, name=f"g_{ht4}", tag="w")
            if ht4 == 0:
                g0 = 1 if micro else 2
                for q in range(KD // g0):
                    dma(nc.sync, gt[:, g0 * q:g0 * q + g0, :],
                        gw[ht4][:, g0 * q:g0 * q + g0, :])
            else:
                dma(nc.sync, gt[:, :, :], gw[ht4][:, :, :])
            ut = wpool.tile([P, KD, 512], BF16, name=f"u_{ht4}", tag="w")
            if ht4 == 0 and x1_pending is not None:
                x1_t, x1_ct = x1_pending
                dma(nc.scalar, ut[:, 0:4, :], uw[ht4][:, 0:4, :])
                dma(nc.scalar, x1_t[:, :, :], xt[x1_ct][:, :, :])
                dma(nc.scalar, ut[:, 4:8, :], uw[ht4][:, 4:8, :])
                for ct in cts[2:]:
                    dma(nc.scalar, xts[ct][:, :, :], xt[ct][:, :, :])
            else:
                dma(nc.scalar, ut[:, :, :], uw[ht4][:, :, :])
            for ci in range(len(cts)):
                tmps = []
                for sub in range(4):
                    ht = ht4 * 4 + sub
                    pg = pspool.tile([P, CT], F32, name=f"pg_{ht}_{ci}", tag="ps")
                    for kt in range(KD):
                        nc.tensor.matmul(
                            pg[:, :],
                            gt[:, kt, sub * P:(sub + 1) * P],
                            xts[ci][:, kt, :],
                            start=(kt == 0),
                            stop=(kt == KD - 1),
                        )
                    tmp = tpool.tile([P, CT], F32, name=f"t_{ht}_{ci}", tag="t")
                    nc.scalar.activation(tmp[:, :], pg[:, :], AF.Silu)
                    tmps.append(tmp)
                for sub in range(4):
                    ht = ht4 * 4 + sub
                    pu = pspool.tile([P, CT], F32, name=f"pu_{ht}_{ci}", tag="ps")
                    for kt in range(KD):
                        nc.tensor.matmul(
                            pu[:, :],
                            ut[:, kt, sub * P:(sub + 1) * P],
                            xts[ci][:, kt, :],
                            start=(kt == 0),
                            stop=(kt == KD - 1),
                        )
                    nc.vector.tensor_mul(
                        hs[ci][ht][:, :], tmps[sub][:, :], pu[:, :]
                    )

        # ---- stage B: dw SBUF-resident, 4 phases of (dc, ci-pair) ----
        dts = {}
        for dc in range(2):
            for hf in range(4):
                dt_ = dpool.tile([P, KH // 4, 512], BF16,
                                 name=f"d_{dc}_{hf}", tag="dw")
                dma(nc.sync, dt_[:, :, :],
                    dw[dc, hf // 2][:, (hf % 2) * 4:(hf % 2) * 4 + 4, :])
                dts[(dc, hf)] = dt_

        for dc in range(2):
            for cis in ((0, 1), (2, 3)):
                pos = {}
                for ci in cis:
                    for m in range(CT // P):
                        pos[(ci, m)] = pspool.tile(
                            [P, 512], F32, name=f"po_{dc}_{ci}_{m}", tag="ps"
                        )
                ots = {
                    ci: opool.tile([P, CT // P, 512], BF16,
                                   name=f"o_{dc}_{ci}", tag="o")
                    for ci in cis
                }
                for hf in range(4):
                    dt_ = dts[(dc, hf)]
                    for kb in range(KH // 4):
                        kh = hf * (KH // 4) + kb
                        for ci in cis:
                            for m in range(CT // P):
                                nc.tensor.matmul(
                                    pos[(ci, m)][:, :],
                                    hs[ci][kh][:, m * P:(m + 1) * P],
                                    dt_[:, kb, :],
                                    start=(kh == 0),
                                    stop=(kh == KH - 1),
                                )
                for ci in cis:
                    ct = ci
                    for m in range(CT // P):
                        j = ct * (CT // P) + m
                        if m % 2 == 0:
                            nc.scalar.mul(
                                ots[ci][:, m, :], pos[(ci, m)][:, :],
                                p_sb[:, j:j + 1]
                            )
                        else:
                            nc.vector.tensor_scalar_mul(
                                ots[ci][:, m, :], pos[(ci, m)][:, :],
                                p_sb[:, j:j + 1]
                            )
                        last_round = dc == 1 and cis == (2, 3)
                        if not dma_out:
                            pass
                        elif store_mode == "gpsimd_half":
                            if last_round:
                                nc.gpsimd.dma_start(
                                    y[ct, dc][:, m:m + 1, :],
                                    ots[ci][:, m:m + 1, :],
                                )
                            elif m % 2 == 1:
                                nc.gpsimd.dma_start(
                                    y[ct, dc][:, m - 1:m + 1, :],
                                    ots[ci][:, m - 1:m + 1, :],
                                )
                        elif last_round and m % 2 == 1:
                            eng = nc.sync if m == 1 else nc.scalar
                            eng.dma_start(
                                y[ct, dc][:, m - 1:m + 1, :],
                                ots[ci][:, m - 1:m + 1, :],
                            )
                        elif m == CT // P - 1:
                            eng = nc.sync if dc == 0 else nc.scalar
                            eng.dma_start(y[ct, dc][:, :, :], ots[ci][:, :, :])


def emit_expert_ffn_v3(tc, xt, gw, uw, dw, pv, y, dma_in=True, dma_out=True,
                       micro=True, store_mode="gpsimd_half", full_reuse=False,
                       reuse_n=2):
    """Baseline body + weight-tile reuse: superblock 1 iterates ht4 groups
    and dc halves in REVERSE and reuses the gate/up tiles of ht4 {3,2} and
    the down tiles of dc=1 still resident in SBUF from superblock 0
    (wpool/dpool bufs 3->4), skipping 6MB of the 24MB per-rep weight DMA.
    Superblock 1's stage-B kh accumulation runs 15..0 so h tiles are
    consumed in the order stage A produces them."""
    nc = tc.nc

    def dma(engine, dst, src):
        if dma_in:
            engine.dma_start(dst, src)
        else:
            ix = (slice(None),) + (slice(0, 1),) * (len(dst.shape) - 1)
            engine.dma_start(dst[ix], src[ix])

    sbs = [list(range(s, min(s + 2, NCT))) for s in range(0, NCT, 2)]
    w_cache = {}
    d_cache = {}

    if full_reuse:
        reuse_n = 4
    wbufs = {2: 4, 3: 6, 4: 8, 5: 8}[reuse_n]
    with (
        tc.tile_pool(name="xpool", bufs=2) as xpool,
        tc.tile_pool(name="wpool", bufs=wbufs) as wpool,
        tc.tile_pool(name="hpool", bufs=36) as hpool,
        tc.tile_pool(name="dpool", bufs=8 if (full_reuse or reuse_n >= 5)
                     else 4) as dpool,
        tc.tile_pool(name="tpool", bufs=5) as tpool,
        tc.tile_pool(name="opool", bufs=2) as opool,
        tc.tile_pool(name="ppool", bufs=1) as ppool,
        tc.tile_pool(name="pspool", bufs=8, space="PSUM") as pspool,
    ):
        p_sb = ppool.tile([P, C // P], F32)
        dma(nc.scalar, p_sb[:, :], pv[:, :])

        for sbi, cts in enumerate(sbs):
            first_sb = sbi == 0
            if full_reuse:
                # all weights SBUF-resident after sb0: forward order both
                # superblocks, sb1 issues zero weight DMAs
                ht4_order = list(range(NH4))
                dc_order = [0, 1]
                reuse_w = set() if first_sb else {0, 1, 2, 3}
                reuse_d = set() if first_sb else {0, 1}
            else:
                ht4_order = (list(range(NH4)) if first_sb
                             else list(range(NH4))[::-1])
                dc_order = [0, 1] if first_sb else [1, 0]
                reuse_w = (set() if first_sb
                           else {3, 2} if reuse_n == 2
                           else {3, 2, 1} if reuse_n == 3
                           else {3, 2, 1, 0})
                reuse_d = (set() if first_sb
                           else {1} if reuse_n < 5 else {1, 0})

            xts = []
            x1_pending = None
            for ct in cts:
                x_t = xpool.tile([P, KD, CT], BF16, name=f"xt_{ct}", tag="xt")
                if ct == 0:
                    g0 = 1 if micro else 2
                    for q in range(KD // g0):
                        dma(nc.scalar, x_t[:, g0 * q:g0 * q + g0, :],
                            xt[ct][:, g0 * q:g0 * q + g0, :])
                elif first_sb:
                    x1_pending = (x_t, ct)
                else:
                    dma(nc.scalar, x_t[:, :, :], xt[ct][:, :, :])
                xts.append(x_t)
            hs = [
                [hpool.tile([P, CT], BF16, name=f"h_{ct}_{ht}", tag="h")
                 for ht in range(KH)]
                for ct in cts
            ]

            # ---- stage A ----
            for ht4 in ht4_order:
                if ht4 in reuse_w:
                    gt, ut = w_cache[ht4]
                else:
                    gt = wpool.tile([P, KD, 512], BF16,
                                    name=f"g_{sbi}_{ht4}", tag="w")
                    if ht4 == 0 and first_sb:
                        g0 = 1 if micro else 2
                        for q in range(KD // g0):
                            dma(nc.sync, gt[:, g0 * q:g0 * q + g0, :],
                                gw[ht4][:, g0 * q:g0 * q + g0, :])
                    else:
                        dma(nc.sync, gt[:, :, :], gw[ht4][:, :, :])
                    ut = wpool.tile([P, KD, 512], BF16,
                                    name=f"u_{sbi}_{ht4}", tag="w")
                    if ht4 == 0 and x1_pending is not None:
                        x1_t, x1_ct = x1_pending
                        dma(nc.scalar, ut[:, 0:4, :], uw[ht4][:, 0:4, :])
                        dma(nc.scalar, x1_t[:, :, :], xt[x1_ct][:, :, :])
                        dma(nc.scalar, ut[:, 4:8, :], uw[ht4][:, 4:8, :])
                    else:
                        dma(nc.scalar, ut[:, :, :], uw[ht4][:, :, :])
                    if first_sb:
                        w_cache[ht4] = (gt, ut)
                for ci in range(len(cts)):
                    tmps = []
                    for sub in range(4):
                        ht = ht4 * 4 + sub
                        pg = pspool.tile([P, CT], F32,
                                         name=f"pg_{sbi}_{ht}_{ci}", tag="ps")
                        for kt in range(KD):
                            nc.tensor.matmul(
                                pg[:, :],
                                gt[:, kt, sub * P:(sub + 1) * P],
                                xts[ci][:, kt, :],
                                start=(kt == 0),
                                stop=(kt == KD - 1),
                            )
                        tmp = tpool.tile([P, CT], F32,
                                         name=f"t_{sbi}_{ht}_{ci}", tag="t")
                        nc.scalar.activation(tmp[:, :], pg[:, :], AF.Silu)
                        tmps.append(tmp)
                    for sub in range(4):
                        ht = ht4 * 4 + sub
                        pu = pspool.tile([P, CT], F32,
                                         name=f"pu_{sbi}_{ht}_{ci}", tag="ps")
                        for kt in range(KD):
                            nc.tensor.matmul(
                                pu[:, :],
                                ut[:, kt, sub * P:(sub + 1) * P],
                                xts[ci][:, kt, :],
                                start=(kt == 0),
                                stop=(kt == KD - 1),
                            )
                        nc.vector.tensor_mul(
                            hs[ci][ht][:, :], tmps[sub][:, :], pu[:, :]
                        )

            # ---- stage B ----
            for dci, dc in enumerate(dc_order):
                pos = {}
                for ci in range(len(cts)):
                    for m in range(CT // P):
                        pos[(ci, m)] = pspool.tile(
                            [P, 512], F32, name=f"po_{sbi}_{dc}_{ci}_{m}",
                            tag="ps"
                        )
                ots = [
                    opool.tile([P, CT // P, 512], BF16,
                               name=f"o_{sbi}_{dc}_{ci}", tag="o")
                    for ci in range(len(cts))
                ]
                fwd = first_sb or full_reuse
                hf_order = list(range(4)) if fwd else list(range(4))[::-1]
                kb_order = (list(range(KH // 4)) if fwd
                            else list(range(KH // 4))[::-1])
                kh_first = 0 if fwd else KH - 1
                kh_last = KH - 1 if fwd else 0
                for hf in hf_order:
                    if dc in reuse_d:
                        dt_ = d_cache[(dc, hf)]
                    elif (dc, hf, sbi) in d_cache:
                        dt_ = d_cache[(dc, hf, sbi)]
                    else:
                        dt_ = dpool.tile([P, KH // 4, 512], BF16,
                                         name=f"d_{sbi}_{dc}_{hf}", tag="dw")
                        dma(nc.sync, dt_[:, :, :],
                            dw[dc, hf // 2][:, (hf % 2) * 4:(hf % 2) * 4 + 4, :])
                        if first_sb:
                            d_cache[(dc, hf)] = dt_
                        else:
                            d_cache[(dc, hf, sbi)] = dt_
                    for kb in kb_order:
                        kh = hf * (KH // 4) + kb
                        for ci in range(len(cts)):
                            for m in range(CT // P):
                                nc.tensor.matmul(
                                    pos[(ci, m)][:, :],
                                    hs[ci][kh][:, m * P:(m + 1) * P],
                                    dt_[:, kb, :],
                                    start=(kh == kh_first),
                                    stop=(kh == kh_last),
                                )
                for ci, ct in enumerate(cts):
                    for m in range(CT // P):
                        j = ct * (CT // P) + m
                        if m % 2 == 0:
                            nc.scalar.mul(
                                ots[ci][:, m, :], pos[(ci, m)][:, :],
                                p_sb[:, j:j + 1]
                            )
                        else:
                            nc.vector.tensor_scalar_mul(
                                ots[ci][:, m, :], pos[(ci, m)][:, :],
                                p_sb[:, j:j + 1]
                            )
                        last_round = (micro and sbi == len(sbs) - 1
                                      and dci == 1)
                        if not dma_out:
                            pass
                        elif last_round:
                            nc.gpsimd.dma_start(
                                y[ct, dc][:, m:m + 1, :],
                                ots[ci][:, m:m + 1, :],
                            )
                        elif m % 2 == 1:
                            nc.gpsimd.dma_start(
                                y[ct, dc][:, m - 1:m + 1, :],
                                ots[ci][:, m - 1:m + 1, :],
                            )


def build_nc(reps_loop=False, max_reps=512, dma_in=True, dma_out=True, unroll=1,
             stagger=False, micro=True, store_mode="gpsimd_half", bodies=1,
             v2=False, v3=True, full_reuse=False, reuse_n=5):
    """Build the per-core Bass program. With reps_loop, the whole body runs
    inside a For_i whose trip count is read from an int32 input "reps"."""
    nc = bacc.Bacc(None, target_bir_lowering=False)

    def emit(tc, xt, gw, uw, dw, pv, y, dma_in=True, dma_out=True, micro=True):
        if v3:
            return emit_expert_ffn_v3(tc, xt, gw, uw, dw, pv, y, dma_in=dma_in,
                                      dma_out=dma_out, micro=micro,
                                      store_mode=store_mode,
                                      full_reuse=full_reuse, reuse_n=reuse_n)
        fn = emit_expert_ffn_v2 if v2 else emit_expert_ffn
        return fn(tc, xt, gw, uw, dw, pv, y, dma_in=dma_in,
                  dma_out=dma_out, micro=micro, store_mode=store_mode)
    with tile.TileContext(nc) as tc:
        xt = nc.dram_tensor("xt", [NCT, P, KD, CT], BF16, kind="ExternalInput")
        gw = nc.dram_tensor("gw", [NH4, P, KD, 512], BF16, kind="ExternalInput")
        uw = nc.dram_tensor("uw", [NH4, P, KD, 512], BF16, kind="ExternalInput")
        pv = nc.dram_tensor("pv", [P, C // P], F32, kind="ExternalInput")
        dw = nc.dram_tensor("dw", [2, 2, P, KH // 2, 512], BF16, kind="ExternalInput")
        y = nc.dram_tensor("y", [NCT, 2, P, CT // P, 512], BF16,
                           kind="ExternalOutput")
        if reps_loop:
            reps = nc.dram_tensor("reps", [1, 1], mybir.dt.int32, kind="ExternalInput")
            with tc.tile_pool(name="rpool", bufs=1) as rpool:
                r_sb = rpool.tile([1, 1], mybir.dt.int32)
                nc.sync.dma_start(r_sb[:, :], reps[:, :])
                rv = nc.values_load(
                    r_sb[0:1, 0:1],
                    min_val=0,
                    max_val=max_reps,
                    skip_runtime_bounds_check=True,
                )
            with tc.For_i(0, rv, 1, staggered_reset=stagger):
                for _ in range(unroll):
                    emit(tc, xt, gw, uw, dw, pv, y,
                         dma_in=dma_in, dma_out=dma_out, micro=micro)
        else:
            for _ in range(bodies):
                emit(tc, xt, gw, uw, dw, pv, y, dma_in=dma_in, dma_out=dma_out,
                     micro=micro)
    nc.compile()
    return nc


def pack_inputs(x_pad, gate_w_e, up_w_e, down_w_e, p_pad):
    """Pack one expert's inputs into the SBUF-tile-order DRAM layouts."""
    # xt [NCT, 128, KD, 512]: [ct, p, kt, tok] = x_pad[ct*512+tok, kt*128+p]
    xt = np.ascontiguousarray(
        x_pad.reshape(NCT, CT, KD, P).transpose(0, 3, 2, 1).astype(NPBF16)
    )
    # gw/uw [NH4, 128, KD, 512]: [b, p, kt, h] = w[b*512+h, kt*128+p]
    gw = np.ascontiguousarray(
        gate_w_e.reshape(NH4, 512, KD, P).transpose(0, 3, 2, 1).astype(NPBF16)
    )
    uw = np.ascontiguousarray(
        up_w_e.reshape(NH4, 512, KD, P).transpose(0, 3, 2, 1).astype(NPBF16)
    )
    # dw [2, 2, 128, KH//2, 512]: [dc, hf, p, kb, d] = down[dc*512+d, hf*1024+kb*128+p]
    dw = np.ascontiguousarray(
        down_w_e.reshape(2, 512, 2, KH // 2, P).transpose(0, 2, 4, 3, 1).astype(NPBF16)
    )
    pv = np.ascontiguousarray(p_pad.reshape(C // P, P).T)
    return {"xt": xt, "gw": gw, "uw": uw, "dw": dw, "pv": pv}


def unpack_y(y_pack):
    """y_pack [NCT, 2, 128, 4, 512] bf16 -> y [C, D] f32."""
    return np.ascontiguousarray(
        y_pack.transpose(0, 3, 2, 1, 4).reshape(C, D).astype(np.float32)
    )


def route_and_dispatch(x, router_w):
    """Host router + top-2 dispatch (matches softmax/top_k/renorm of the
    reference exactly)."""
    logits = x @ router_w.T                      # [T, E]
    t_ar = np.arange(T)
    i1 = np.argmax(logits, axis=1)
    l1 = logits[t_ar, i1]
    lm = logits.copy()
    lm[t_ar, i1] = -np.inf
    i2 = np.argmax(lm, axis=1)
    l2 = lm[t_ar, i2]
    e2 = np.exp(l2 - l1)
    p1 = 1.0 / (1.0 + e2)
    p2 = e2 / (1.0 + e2)

    ee = np.concatenate([i1, i2])                # [2T] expert of each pair
    tt = np.concatenate([t_ar, t_ar])            # [2T] token of each pair
    pp = np.concatenate([p1, p2]).astype(np.float32)
    counts = np.bincount(ee, minlength=E)
    starts = np.zeros(E, np.int64)
    starts[1:] = np.cumsum(counts)[:-1]
    order = np.argsort(ee, kind="stable")
    pos = np.empty(2 * T, np.int64)
    pos[order] = np.arange(2 * T) - starts[ee[order]]
    return ee, tt, pp, pos, counts, starts, order


def kernel(**inputs):
    x = np.ascontiguousarray(
        np.asarray(inputs["hidden_states"], np.float32).reshape(T, D)
    )
    router_w = np.asarray(inputs["router_w"], np.float32)
    gate_w = np.asarray(inputs["gate_w"], np.float32)
    up_w = np.asarray(inputs["up_w"], np.float32)
    down_w = np.asarray(inputs["down_w"], np.float32)

    ee, tt, pp, pos, counts, starts, order = route_and_dispatch(x, router_w)

    in_maps = []
    for e in range(E):
        n_e = min(int(counts[e]), C)
        sel = order[starts[e]:starts[e] + n_e]   # pairs dispatched to core e
        xp = np.zeros((C, D), np.float32)
        xp[:n_e] = x[tt[sel]]
        pvec = np.zeros(C, np.float32)
        pvec[:n_e] = pp[sel]
        in_maps.append(pack_inputs(xp, gate_w[e], up_w[e], down_w[e], pvec))

    nc = build_nc()
    res = run_bass_kernel_spmd(nc, in_maps, core_ids=list(range(E)))
    ys = np.stack(
        [unpack_y(res.results[e]["y"]) for e in range(E)]
    ).reshape(E * C, D)

    ok = pos < C
    contrib = np.zeros((2 * T, D), np.float32)
    g = ee * C + pos
    contrib[ok] = ys[g[ok]]
    # capacity-overflow fallback: exact fp32 host compute for the few pairs
    # beyond capacity (~0.8% of pairs for the seed-0 routing), batched per
    # expert
    if not ok.all():
        bad = np.nonzero(~ok)[0]
        for e in np.unique(ee[bad]):
            js = bad[ee[bad] == e]
            xb = x[tt[js]]
            gb = xb @ gate_w[e].T
            ub = xb @ up_w[e].T
            hb = (gb / (1.0 + np.exp(-gb))) * ub
            contrib[js] = (hb @ down_w[e].T) * pp[js, None]
    out = contrib[:T] + contrib[T:]
    return out.reshape(B, S, D).astype(np.float32)

